# revision 6
# baseline (speedup 1.0000x reference)
"""Trainium2 Bass kernel for PointCloudFitter (brute-force 1-NN min distance).

reference semantics:
    R = so3_exp_map(rot); transformed = einsum('ij,bnj->bni', R, source) + trans
    d2[b,n,m] = ||transformed[b,n] - target[b,m]||^2
    returns (transformed [B,N,3] f32, loss = mean over (b,n) of min_m d2)

Strategy:
  - Host computes the tiny transform (R @ p + t) and packs, per core,
      q_pack [5, 2048] = [x0, x1, x2, ||x||^2, 1]      (queries, lhsT layout)
      t_pack [5, 4096] = [-2y0, -2y1, -2y2, 1, ||y||^2] (targets, rhs layout)
    so that the PE matmul  q_pack.T @ t_pack  directly yields d2 >= 0.
  - 8 cores = (4 batches) x (2 halves of the N axis). Each core computes
    min over all M=4096 targets for its 2048 queries:
    16 n-tiles x [K=5,128] weights, streaming targets in 512-wide chunks into
    PSUM, DVE reduce(min) over [128, 2048] chunks.
  - Host gathers per-core [128, 16, 2] partial mins, takes the final min and
    the mean in float64.
"""

import numpy as np

import concourse.bass as bass
import concourse.mybir as mybir
import concourse.tile as tile
from concourse import bacc
from concourse.bass_utils import run_bass_kernel_spmd

B, N, M = 4, 4096, 4096
NCORES = 8
NQ = N // 2  # queries per core
NTILES = NQ // 128  # 16
MCHUNK = 2048  # PSUM chunk: 4 banks of 512 f32
NCHUNKS = M // MCHUNK  # 2
KDIM = 5
F32 = mybir.dt.float32


def _build_nc():
    # Bacc (not raw Bass): its compile passes split multi-waits
    # (move_matmul_waits_to_ldweights / generate_event_semaphores) that
    # walrus' S3_LW struct can't encode.
    nc = bacc.Bacc(trn_type="TRN2")
    # Single input tensor (one DMA → one semaphore): LDWEIGHTS-carrying
    # matmuls only support a single sync-wait command in walrus codegen.
    qt_d = nc.dram_tensor("qt_pack", [KDIM, NQ + M], F32, kind="ExternalInput")
    out_d = nc.dram_tensor("mins", [128, NTILES, NCHUNKS], F32, kind="ExternalOutput")

    with tile.TileContext(nc) as tc:
        with (
            tc.tile_pool(name="io", bufs=1) as io_pool,
            tc.tile_pool(name="res", bufs=1) as res_pool,
            tc.tile_pool(name="ps", bufs=2, space="PSUM") as ps_pool,
        ):
            qt = io_pool.tile([KDIM, NQ + M], F32)
            nc.sync.dma_start(out=qt[:], in_=qt_d[:])
            q = qt[:, :NQ]
            t = qt[:, NQ:]

            partials = res_pool.tile([128, NTILES, NCHUNKS], F32)

            for nt in range(NTILES):
                qw = q[:, nt * 128 : (nt + 1) * 128]
                for h in range(NCHUNKS):
                    ps = ps_pool.tile([128, MCHUNK], F32)
                    for j in range(MCHUNK // 512):
                        nc.tensor.matmul(
                            ps[:, j * 512 : (j + 1) * 512],
                            qw,
                            t[:, h * MCHUNK + j * 512 : h * MCHUNK + (j + 1) * 512],
                            start=True,
                            stop=True,
                        )
                    nc.vector.tensor_reduce(
                        out=partials[:, nt, h : h + 1],
                        in_=ps[:],
                        axis=mybir.AxisListType.X,
                        op=mybir.AluOpType.min,
                    )

            nc.sync.dma_start(out=out_d[:], in_=partials[:])
    if not nc.is_finalized():
        nc.finalize()
    return nc


_NC_CACHE = {}


def _get_nc():
    if "nc" not in _NC_CACHE:
        _NC_CACHE["nc"] = _build_nc()
    return _NC_CACHE["nc"]


def _host_transform(source_pcd, rot, trans):
    """Rodrigues + affine transform, in float64 for accuracy."""
    w = rot.astype(np.float64)
    theta2 = float(np.dot(w, w))
    eps = 1e-8
    theta = np.sqrt(theta2 + eps)
    A = np.sin(theta) / theta
    Bc = (1.0 - np.cos(theta)) / (theta2 + eps)
    wx, wy, wz = w
    hat = np.array([[0.0, -wz, wy], [wz, 0.0, -wx], [-wy, wx, 0.0]])
    R = np.eye(3) + A * hat + Bc * (hat @ hat)
    tf = source_pcd.astype(np.float64) @ R.T + trans.astype(np.float64)
    return tf.astype(np.float32)


def _pack_inputs(transformed, target_pcd):
    """Build per-core input maps."""
    in_maps = []
    for b in range(B):
        X = transformed[b].astype(np.float32)  # [N,3]
        Y = target_pcd[b].astype(np.float32)  # [M,3]
        x2 = np.sum(X.astype(np.float64) ** 2, axis=1).astype(np.float32)
        y2 = np.sum(Y.astype(np.float64) ** 2, axis=1).astype(np.float32)
        qf = np.stack(
            [X[:, 0], X[:, 1], X[:, 2], x2, np.ones(N, np.float32)]
        )  # [5, N]
        tf_ = np.stack(
            [-2.0 * Y[:, 0], -2.0 * Y[:, 1], -2.0 * Y[:, 2], np.ones(M, np.float32), y2]
        ).astype(np.float32)  # [5, M]
        for h in range(2):
            qt = np.concatenate([qf[:, h * NQ : (h + 1) * NQ], tf_], axis=1)
            in_maps.append({"qt_pack": np.ascontiguousarray(qt)})
    return in_maps


def _run(inputs, trace=False, trace_kwargs=None):
    """Returns ((transformed, loss), BassKernelResults)."""
    source_pcd = np.asarray(inputs["source_pcd"], dtype=np.float32)
    target_pcd = np.asarray(inputs["target_pcd"], dtype=np.float32)
    rot = np.asarray(inputs["rot"], dtype=np.float32)
    trans = np.asarray(inputs["trans"], dtype=np.float32)

    transformed = _host_transform(source_pcd, rot, trans)
    in_maps = _pack_inputs(transformed, target_pcd)

    nc = _get_nc()
    br = run_bass_kernel_spmd(
        nc,
        in_maps,
        core_ids=list(range(NCORES)),
        trace=trace,
        **(trace_kwargs or {}),
    )

    mins_per_query = np.empty((B, N), dtype=np.float64)
    for c in range(NCORES):
        b, h = divmod(c, 2)
        part = np.asarray(br.results[c]["mins"], dtype=np.float64)  # [128,16,2]
        v = part.min(axis=-1)  # [128, 16] ; query q = t*128 + p  -> v[p, t]
        mins_per_query[b, h * NQ : (h + 1) * NQ] = v.T.reshape(NQ)

    loss = np.float32(mins_per_query.mean())
    return (transformed, loss), br


def kernel(**inputs):
    (transformed, loss), _ = _run(inputs, trace=False)
    return (transformed, loss)


# revision 23
# speedup vs baseline: 2.7846x; 2.7846x over previous
"""Trainium2 Bass kernel for PointCloudFitter (brute-force 1-NN min distance).

reference semantics:
    R = so3_exp_map(rot); transformed = einsum('ij,bnj->bni', R, source) + trans
    d2[b,n,m] = ||transformed[b,n] - target[b,m]||^2
    returns (transformed [B,N,3] f32, loss = mean over (b,n) of min_m d2)

Strategy:
  - Host computes the tiny transform (R @ p + t) and packs, per core,
      q_pack [5, 2048] = [x0, x1, x2, ||x||^2, 1]      (queries, lhsT layout)
      t_pack [5, 4096] = [-2y0, -2y1, -2y2, 1, ||y||^2] (targets, rhs layout)
    so that the PE matmul  q_pack.T @ t_pack  directly yields d2 >= 0.
  - 8 cores = (4 batches) x (2 halves of the N axis). Each core computes
    min over all M=4096 targets for its 2048 queries:
    16 n-tiles x [K=5,128] weights, streaming targets in 512-wide chunks into
    PSUM, DVE reduce(min) over [128, 2048] chunks.
  - Host gathers per-core [128, 16, 2] partial mins, takes the final min and
    the mean in float64.
"""

import numpy as np

import concourse.bass as bass
import concourse.mybir as mybir
import concourse.tile as tile
from concourse import bacc
from concourse.bass_utils import run_bass_kernel_spmd

B, N, M = 4, 4096, 4096
NCORES = 8
NQ = N // 2  # queries per core
NTILES = NQ // 128  # 16
MCHUNK = 2048  # PSUM chunk: 4 banks of 512 f32
NCHUNKS = M // MCHUNK  # 2
KDIM = 5
F32 = mybir.dt.float32
# float32r: single-pass fp32 matmul (1 cycle/row at free-dim >= 256) vs
# regular fp32's LOW_HIGH 2-pass at 2 cycles/row each.
F32R = mybir.dt.float32r


BF16 = mybir.dt.bfloat16
N_DIRECT = 2  # PSUM banks per n-tile reduced directly by DVE (fp32-exact)
N_COPIED = 8 - N_DIRECT  # banks ACT-copied to bf16 SBUF, then DVE TTR-min
NPART = N_DIRECT + 1  # partial-min columns per n-tile
NGROUPS = 1  # PE row groups used for matmul rotation (1 = no rotation)


def _build_nc(mm_dtype=F32R, reduce_mode="split_tt", ngroups=NGROUPS):
    # Bacc (not raw Bass): its compile passes split multi-waits
    # (move_matmul_waits_to_ldweights / generate_event_semaphores) that
    # walrus' S3_LW struct can't encode.
    nc = bacc.Bacc(trn_type="TRN2")
    npart = 8 if reduce_mode == "dve" else NPART
    # Single input tensor (one DMA → one semaphore): LDWEIGHTS-carrying
    # matmuls only support a single sync-wait command in walrus codegen.
    qt_d = nc.dram_tensor("qt_pack", [KDIM, NQ + M], mm_dtype, kind="ExternalInput")
    out_d = nc.dram_tensor("mins", [128, NTILES, npart], F32, kind="ExternalOutput")

    with tile.TileContext(nc) as tc:
        with (
            tc.tile_pool(name="io", bufs=1) as io_pool,
            tc.tile_pool(name="res", bufs=1) as res_pool,
            tc.tile_pool(name="stage", bufs=2) as stage_pool,
            tc.tile_pool(name="scr", bufs=2) as scr_pool,
            tc.tile_pool(name="ps", bufs=8, space="PSUM") as ps_pool,
        ):
            # Replicate the [5, NQ+M] pack into NGROUPS PE row groups
            # (partition offsets 0/32/64/96) so consecutive matmuls target
            # different row groups: their LDWEIGHTS overlap in-flight
            # MATMULs and the MMs run concurrently on the 32-row subarrays.
            qt = io_pool.tile([32 * (ngroups - 1) + KDIM, NQ + M], mm_dtype)
            for g in range(ngroups):
                nc.sync.dma_start(
                    out=qt[32 * g : 32 * g + KDIM, :], in_=qt_d[:]
                )

            partials = res_pool.tile([128, NTILES, npart], F32)

            for nt in range(NTILES):
                banks = []
                for j in range(8):
                    g = j % ngroups
                    rows = slice(32 * g, 32 * g + KDIM)
                    ps = ps_pool.tile([128, 512], F32)
                    nc.tensor.matmul(
                        ps[:],
                        qt[rows, nt * 128 : (nt + 1) * 128],
                        qt[rows, NQ + j * 512 : NQ + (j + 1) * 512],
                        start=True,
                        stop=True,
                        tile_position=(32 * g, 0) if ngroups > 1 else None,
                    )
                    banks.append(ps)

                if reduce_mode == "dve":
                    for dd in range(8):
                        nc.vector.tensor_reduce(
                            out=partials[:, nt, dd : dd + 1],
                            in_=banks[dd][:],
                            axis=mybir.AxisListType.X,
                            op=mybir.AluOpType.min,
                        )
                    continue

                # DVE: direct fp32 min-reduce of the first N_DIRECT banks.
                for dd in range(N_DIRECT):
                    nc.vector.tensor_reduce(
                        out=partials[:, nt, dd : dd + 1],
                        in_=banks[dd][:],
                        axis=mybir.AxisListType.X,
                        op=mybir.AluOpType.min,
                    )
                # ACT: copy the remaining banks into an SBUF stage
                # (d2 is exact >= 0 and small near minima, so bf16 is safe).
                stage_dt = BF16 if reduce_mode in ("split", "split_tt") else F32
                stage = stage_pool.tile([128, N_COPIED * 512], stage_dt)
                for a in range(N_COPIED):
                    nc.scalar.copy(
                        out=stage[:, a * 512 : (a + 1) * 512],
                        in_=banks[N_DIRECT + a][:],
                    )
                half = N_COPIED * 512 // 2
                if reduce_mode == "split_tt":
                    # TENSOR_TENSOR_REDUCE hangs TRN2 hardware (bisected:
                    # crashes with min/min at any dtype), so use an explicit
                    # tensor_tensor min tree (bf16 SBUF = 2x mode) + a short
                    # final 1x reduce instead.
                    width = half
                    scr = scr_pool.tile([128, width], BF16)
                    nc.vector.tensor_tensor(
                        out=scr[:],
                        in0=stage[:, :width],
                        in1=stage[:, width:],
                        op=mybir.AluOpType.min,
                    )
                    while width > 384:
                        nw = width // 2
                        nc.vector.tensor_tensor(
                            out=scr[:, :nw],
                            in0=scr[:, :nw],
                            in1=scr[:, nw:width],
                            op=mybir.AluOpType.min,
                        )
                        width = nw
                    nc.vector.tensor_reduce(
                        out=partials[:, nt, N_DIRECT : N_DIRECT + 1],
                        in_=scr[:, :width],
                        axis=mybir.AxisListType.X,
                        op=mybir.AluOpType.min,
                    )
                else:
                    # DVE: fused pairwise-min + min-reduce over the stage
                    # (bf16 tensor_tensor runs in 2x mode from SBUF).
                    scr = scr_pool.tile([128, half], stage_dt)
                    nc.vector.tensor_tensor_reduce(
                        out=scr[:],
                        in0=stage[:, :half],
                        in1=stage[:, half:],
                        scale=1.0,
                        scalar=3.0e38,
                        op0=mybir.AluOpType.min,
                        op1=mybir.AluOpType.min,
                        accum_out=partials[:, nt, N_DIRECT : N_DIRECT + 1],
                    )

            nc.sync.dma_start(out=out_d[:], in_=partials[:])
    if not nc.is_finalized():
        nc.finalize()
    return nc


_NC_CACHE = {}


def _get_nc(**kw):
    key = tuple(sorted(kw.items()))
    if key not in _NC_CACHE:
        _NC_CACHE[key] = _build_nc(**kw)
    return _NC_CACHE[key]


def _host_transform(source_pcd, rot, trans):
    """Rodrigues + affine transform, in float64 for accuracy."""
    w = rot.astype(np.float64)
    theta2 = float(np.dot(w, w))
    eps = 1e-8
    theta = np.sqrt(theta2 + eps)
    A = np.sin(theta) / theta
    Bc = (1.0 - np.cos(theta)) / (theta2 + eps)
    wx, wy, wz = w
    hat = np.array([[0.0, -wz, wy], [wz, 0.0, -wx], [-wy, wx, 0.0]])
    R = np.eye(3) + A * hat + Bc * (hat @ hat)
    tf = source_pcd.astype(np.float64) @ R.T + trans.astype(np.float64)
    return tf.astype(np.float32)


def _pack_inputs(transformed, target_pcd):
    """Build per-core input maps."""
    in_maps = []
    for b in range(B):
        X = transformed[b].astype(np.float32)  # [N,3]
        Y = target_pcd[b].astype(np.float32)  # [M,3]
        x2 = np.sum(X.astype(np.float64) ** 2, axis=1).astype(np.float32)
        y2 = np.sum(Y.astype(np.float64) ** 2, axis=1).astype(np.float32)
        qf = np.stack(
            [X[:, 0], X[:, 1], X[:, 2], x2, np.ones(N, np.float32)]
        )  # [5, N]
        tf_ = np.stack(
            [-2.0 * Y[:, 0], -2.0 * Y[:, 1], -2.0 * Y[:, 2], np.ones(M, np.float32), y2]
        ).astype(np.float32)  # [5, M]
        for h in range(2):
            qt = np.concatenate([qf[:, h * NQ : (h + 1) * NQ], tf_], axis=1)
            in_maps.append({"qt_pack": np.ascontiguousarray(qt)})
    return in_maps


def _run(inputs, trace=False, trace_kwargs=None, build_kwargs=None):
    """Returns ((transformed, loss), BassKernelResults)."""
    source_pcd = np.asarray(inputs["source_pcd"], dtype=np.float32)
    target_pcd = np.asarray(inputs["target_pcd"], dtype=np.float32)
    rot = np.asarray(inputs["rot"], dtype=np.float32)
    trans = np.asarray(inputs["trans"], dtype=np.float32)

    transformed = _host_transform(source_pcd, rot, trans)
    in_maps = _pack_inputs(transformed, target_pcd)

    nc = _get_nc(**(build_kwargs or {}))
    br = run_bass_kernel_spmd(
        nc,
        in_maps,
        core_ids=list(range(NCORES)),
        trace=trace,
        **(trace_kwargs or {}),
    )

    mins_per_query = np.empty((B, N), dtype=np.float64)
    for c in range(NCORES):
        b, h = divmod(c, 2)
        part = np.asarray(br.results[c]["mins"], dtype=np.float64)  # [128,16,2]
        v = part.min(axis=-1)  # [128, 16] ; query q = t*128 + p  -> v[p, t]
        mins_per_query[b, h * NQ : (h + 1) * NQ] = v.T.reshape(NQ)

    loss = np.float32(mins_per_query.mean())
    return (transformed, loss), br


def kernel(**inputs):
    (transformed, loss), _ = _run(inputs, trace=False)
    return (transformed, loss)


# revision 27
# speedup vs baseline: 2.9323x; 1.0530x over previous
"""Trainium2 Bass kernel for PointCloudFitter (brute-force 1-NN min distance).

reference semantics:
    R = so3_exp_map(rot); transformed = einsum('ij,bnj->bni', R, source) + trans
    d2[b,n,m] = ||transformed[b,n] - target[b,m]||^2
    returns (transformed [B,N,3] f32, loss = mean over (b,n) of min_m d2)

Strategy:
  - Host computes the tiny transform (R @ p + t) and packs, per core,
      q_pack [5, 2048] = [x0, x1, x2, ||x||^2, 1]      (queries, lhsT layout)
      t_pack [5, 4096] = [-2y0, -2y1, -2y2, 1, ||y||^2] (targets, rhs layout)
    so that the PE matmul  q_pack.T @ t_pack  directly yields d2 >= 0.
  - 8 cores = (4 batches) x (2 halves of the N axis). Each core computes
    min over all M=4096 targets for its 2048 queries:
    16 n-tiles x [K=5,128] weights, streaming targets in 512-wide chunks into
    PSUM, DVE reduce(min) over [128, 2048] chunks.
  - Host gathers per-core [128, 16, 2] partial mins, takes the final min and
    the mean in float64.
"""

import numpy as np

import concourse.bass as bass
import concourse.mybir as mybir
import concourse.tile as tile
from concourse import bacc
from concourse.bass_utils import run_bass_kernel_spmd

B, N, M = 4, 4096, 4096
NCORES = 8
NQ = N // 2  # queries per core
NTILES = NQ // 128  # 16
MCHUNK = 2048  # PSUM chunk: 4 banks of 512 f32
NCHUNKS = M // MCHUNK  # 2
KDIM = 5
F32 = mybir.dt.float32
# float32r: single-pass fp32 matmul (1 cycle/row at free-dim >= 256) vs
# regular fp32's LOW_HIGH 2-pass at 2 cycles/row each.
F32R = mybir.dt.float32r


BF16 = mybir.dt.bfloat16
N_DIRECT = 2  # PSUM banks per n-tile reduced directly by DVE (fp32-exact)
N_COPIED = 8 - N_DIRECT  # banks ACT-copied to bf16 SBUF, then DVE TTR-min
NPART = N_DIRECT + 1  # partial-min columns per n-tile
NGROUPS = 1  # PE row groups used for matmul rotation (1 = no rotation)


def _build_nc(mm_dtype=F32R, reduce_mode="split_tt", ngroups=NGROUPS):
    # Bacc (not raw Bass): its compile passes split multi-waits
    # (move_matmul_waits_to_ldweights / generate_event_semaphores) that
    # walrus' S3_LW struct can't encode.
    nc = bacc.Bacc(trn_type="TRN2")
    npart = {"dve": 8, "v5": 2}.get(reduce_mode, NPART)
    # Single input tensor (one DMA → one semaphore): LDWEIGHTS-carrying
    # matmuls only support a single sync-wait command in walrus codegen.
    qt_d = nc.dram_tensor("qt_pack", [KDIM, NQ + M], mm_dtype, kind="ExternalInput")
    out_d = nc.dram_tensor("mins", [128, NTILES, npart], F32, kind="ExternalOutput")

    with tile.TileContext(nc) as tc:
        with (
            tc.tile_pool(name="io", bufs=1) as io_pool,
            tc.tile_pool(name="res", bufs=1) as res_pool,
            tc.tile_pool(name="stage", bufs=2) as stage_pool,
            tc.tile_pool(name="scr", bufs=2) as scr_pool,
            tc.tile_pool(name="ps", bufs=8, space="PSUM") as ps_pool,
            tc.tile_pool(name="psd", bufs=2, space="PSUM") as psd_pool,
            tc.tile_pool(name="psa", bufs=2, space="PSUM") as psa_pool,
        ):
            # Replicate the [5, NQ+M] pack into NGROUPS PE row groups
            # (partition offsets 0/32/64/96) so consecutive matmuls target
            # different row groups: their LDWEIGHTS overlap in-flight
            # MATMULs and the MMs run concurrently on the 32-row subarrays.
            qt = io_pool.tile([32 * (ngroups - 1) + KDIM, NQ + M], mm_dtype)
            for g in range(ngroups):
                # split replica loads across both HWDGE engines (SP + ACT)
                # so they go out on parallel DMA queues
                eng = nc.sync if g % 2 == 0 else nc.scalar
                eng.dma_start(out=qt[32 * g : 32 * g + KDIM, :], in_=qt_d[:])

            partials = res_pool.tile([128, NTILES, npart], F32)

            def mm(out_ap, j, nt):
                g = j % ngroups
                rows = slice(32 * g, 32 * g + KDIM)
                nc.tensor.matmul(
                    out_ap,
                    qt[rows, nt * 128 : (nt + 1) * 128],
                    qt[rows, NQ + j * 512 : NQ + (j + 1) * 512],
                    start=True,
                    stop=True,
                    tile_position=(32 * g, 0) if ngroups > 1 else None,
                )

            if reduce_mode == "v5":
                # kilotile layout: per n-tile, 3 ACT-copied [128,1024] psum
                # tiles (j=0..5) + 1 DVE-direct [128,1024] psum tile (j=6,7).
                for nt in range(NTILES):
                    stage = stage_pool.tile([128, 3072], BF16)
                    for i in range(3):
                        pa = psa_pool.tile([128, 1024], F32)
                        mm(pa[:, :512], 2 * i, nt)
                        mm(pa[:, 512:], 2 * i + 1, nt)
                        nc.scalar.copy(
                            out=stage[:, i * 1024 : (i + 1) * 1024], in_=pa[:]
                        )
                    pd = psd_pool.tile([128, 1024], F32)
                    mm(pd[:, :512], 6, nt)
                    mm(pd[:, 512:], 7, nt)
                    nc.vector.tensor_reduce(
                        out=partials[:, nt, 0:1],
                        in_=pd[:],
                        axis=mybir.AxisListType.X,
                        op=mybir.AluOpType.min,
                    )
                    width = 1536
                    scr = scr_pool.tile([128, width], BF16)
                    nc.vector.tensor_tensor(
                        out=scr[:],
                        in0=stage[:, :width],
                        in1=stage[:, width:],
                        op=mybir.AluOpType.min,
                    )
                    while width > 384:
                        nw = width // 2
                        nc.vector.tensor_tensor(
                            out=scr[:, :nw],
                            in0=scr[:, :nw],
                            in1=scr[:, nw:width],
                            op=mybir.AluOpType.min,
                        )
                        width = nw
                    nc.vector.tensor_reduce(
                        out=partials[:, nt, 1:2],
                        in_=scr[:, :width],
                        axis=mybir.AxisListType.X,
                        op=mybir.AluOpType.min,
                    )
                nt_iter = []
            else:
                nt_iter = list(range(NTILES))

            for nt in nt_iter:
                banks = []
                for j in range(8):
                    ps = ps_pool.tile([128, 512], F32)
                    mm(ps[:], j, nt)
                    banks.append(ps)

                if reduce_mode == "dve":
                    for dd in range(8):
                        nc.vector.tensor_reduce(
                            out=partials[:, nt, dd : dd + 1],
                            in_=banks[dd][:],
                            axis=mybir.AxisListType.X,
                            op=mybir.AluOpType.min,
                        )
                    continue

                # DVE: direct fp32 min-reduce of the first N_DIRECT banks.
                for dd in range(N_DIRECT):
                    nc.vector.tensor_reduce(
                        out=partials[:, nt, dd : dd + 1],
                        in_=banks[dd][:],
                        axis=mybir.AxisListType.X,
                        op=mybir.AluOpType.min,
                    )
                # ACT: copy the remaining banks into an SBUF stage
                # (d2 is exact >= 0 and small near minima, so bf16 is safe).
                stage_dt = BF16 if reduce_mode in ("split", "split_tt") else F32
                stage = stage_pool.tile([128, N_COPIED * 512], stage_dt)
                for a in range(N_COPIED):
                    nc.scalar.copy(
                        out=stage[:, a * 512 : (a + 1) * 512],
                        in_=banks[N_DIRECT + a][:],
                    )
                half = N_COPIED * 512 // 2
                if reduce_mode == "split_tt":
                    # TENSOR_TENSOR_REDUCE hangs TRN2 hardware (bisected:
                    # crashes with min/min at any dtype), so use an explicit
                    # tensor_tensor min tree (bf16 SBUF = 2x mode) + a short
                    # final 1x reduce instead.
                    width = half
                    scr = scr_pool.tile([128, width], BF16)
                    nc.vector.tensor_tensor(
                        out=scr[:],
                        in0=stage[:, :width],
                        in1=stage[:, width:],
                        op=mybir.AluOpType.min,
                    )
                    while width > 384:
                        nw = width // 2
                        nc.vector.tensor_tensor(
                            out=scr[:, :nw],
                            in0=scr[:, :nw],
                            in1=scr[:, nw:width],
                            op=mybir.AluOpType.min,
                        )
                        width = nw
                    nc.vector.tensor_reduce(
                        out=partials[:, nt, N_DIRECT : N_DIRECT + 1],
                        in_=scr[:, :width],
                        axis=mybir.AxisListType.X,
                        op=mybir.AluOpType.min,
                    )
                else:
                    # DVE: fused pairwise-min + min-reduce over the stage
                    # (bf16 tensor_tensor runs in 2x mode from SBUF).
                    scr = scr_pool.tile([128, half], stage_dt)
                    nc.vector.tensor_tensor_reduce(
                        out=scr[:],
                        in0=stage[:, :half],
                        in1=stage[:, half:],
                        scale=1.0,
                        scalar=3.0e38,
                        op0=mybir.AluOpType.min,
                        op1=mybir.AluOpType.min,
                        accum_out=partials[:, nt, N_DIRECT : N_DIRECT + 1],
                    )

            nc.sync.dma_start(out=out_d[:], in_=partials[:])
    if not nc.is_finalized():
        nc.finalize()
    return nc


_NC_CACHE = {}


def _get_nc(**kw):
    key = tuple(sorted(kw.items()))
    if key not in _NC_CACHE:
        _NC_CACHE[key] = _build_nc(**kw)
    return _NC_CACHE[key]


def _host_transform(source_pcd, rot, trans):
    """Rodrigues + affine transform, in float64 for accuracy."""
    w = rot.astype(np.float64)
    theta2 = float(np.dot(w, w))
    eps = 1e-8
    theta = np.sqrt(theta2 + eps)
    A = np.sin(theta) / theta
    Bc = (1.0 - np.cos(theta)) / (theta2 + eps)
    wx, wy, wz = w
    hat = np.array([[0.0, -wz, wy], [wz, 0.0, -wx], [-wy, wx, 0.0]])
    R = np.eye(3) + A * hat + Bc * (hat @ hat)
    tf = source_pcd.astype(np.float64) @ R.T + trans.astype(np.float64)
    return tf.astype(np.float32)


def _pack_inputs(transformed, target_pcd):
    """Build per-core input maps."""
    in_maps = []
    for b in range(B):
        X = transformed[b].astype(np.float32)  # [N,3]
        Y = target_pcd[b].astype(np.float32)  # [M,3]
        x2 = np.sum(X.astype(np.float64) ** 2, axis=1).astype(np.float32)
        y2 = np.sum(Y.astype(np.float64) ** 2, axis=1).astype(np.float32)
        qf = np.stack(
            [X[:, 0], X[:, 1], X[:, 2], x2, np.ones(N, np.float32)]
        )  # [5, N]
        tf_ = np.stack(
            [-2.0 * Y[:, 0], -2.0 * Y[:, 1], -2.0 * Y[:, 2], np.ones(M, np.float32), y2]
        ).astype(np.float32)  # [5, M]
        for h in range(2):
            qt = np.concatenate([qf[:, h * NQ : (h + 1) * NQ], tf_], axis=1)
            in_maps.append({"qt_pack": np.ascontiguousarray(qt)})
    return in_maps


def _run(inputs, trace=False, trace_kwargs=None, build_kwargs=None):
    """Returns ((transformed, loss), BassKernelResults)."""
    source_pcd = np.asarray(inputs["source_pcd"], dtype=np.float32)
    target_pcd = np.asarray(inputs["target_pcd"], dtype=np.float32)
    rot = np.asarray(inputs["rot"], dtype=np.float32)
    trans = np.asarray(inputs["trans"], dtype=np.float32)

    transformed = _host_transform(source_pcd, rot, trans)
    in_maps = _pack_inputs(transformed, target_pcd)

    nc = _get_nc(**(build_kwargs or {}))
    br = run_bass_kernel_spmd(
        nc,
        in_maps,
        core_ids=list(range(NCORES)),
        trace=trace,
        **(trace_kwargs or {}),
    )

    mins_per_query = np.empty((B, N), dtype=np.float64)
    for c in range(NCORES):
        b, h = divmod(c, 2)
        part = np.asarray(br.results[c]["mins"], dtype=np.float64)  # [128,16,2]
        v = part.min(axis=-1)  # [128, 16] ; query q = t*128 + p  -> v[p, t]
        mins_per_query[b, h * NQ : (h + 1) * NQ] = v.T.reshape(NQ)

    loss = np.float32(mins_per_query.mean())
    return (transformed, loss), br


def kernel(**inputs):
    (transformed, loss), _ = _run(inputs, trace=False)
    return (transformed, loss)
